# revision 3
# baseline (speedup 1.0000x reference)
"""Fused LinearCrossEntropyKL loss on 8 Trainium2 cores (vocab-parallel).

loss = sum_b ce_c[b]*(s_lse[b] - s_logit[b,tgt[b]]) + kl_c[b]*KL(t||s)[b]

Device (per core, vocab shard Vs=6288):
  a_s[b] = sum_v exp(s_logit),  a_t[b] = sum_v exp(t_logit)
  w1[b]  = sum_v exp(t)*t,      w2[b]  = sum_v exp(t)*s
Host combine: s_lse = log(sum_k a_s), u = w1-w2,
  KL = u/a_t - t_lse + s_lse, ce = s_lse - <h_b, c_tgt_b>  (host fp64 dot)
No max-stabilizer: inputs are scaled 1/sqrt(D) so |logit| < ~0.5 and
exp() is safely in range.
"""

import sys

sys.path.insert(0, "/opt/trn_rl_repo")

import numpy as np
import ml_dtypes

import concourse.bacc as bacc
import concourse.bass as bass
import concourse.mybir as mybir
from concourse import tile
from concourse.bass_utils import run_bass_kernel_spmd

B, D, V = 2048, 512, 50304
NCORES = 8
VS = V // NCORES            # 6288 per core
KT = D // 128               # 4 k-tiles
BT = B // 128               # 16 b-tiles
MC_W = 1024                 # megachunk width (2 PSUM banks)
MCS = [MC_W] * (VS // MC_W) + ([VS % MC_W] if VS % MC_W else [])  # 6x1024+144
NMC = len(MCS)

F32 = mybir.dt.float32
BF16 = mybir.dt.bfloat16
Exp = mybir.ActivationFunctionType.Exp
Alu = mybir.AluOpType

_COMPILED = {}


def _build():
    nc = bacc.Bacc(
        "TRN2", target_bir_lowering=False, debug=False, num_devices=NCORES
    )
    hs = nc.dram_tensor("hs", [D, B], BF16, kind="ExternalInput").ap()
    ht = nc.dram_tensor("ht", [D, B], BF16, kind="ExternalInput").ap()
    cs = nc.dram_tensor("cs", [D, VS], BF16, kind="ExternalInput").ap()
    ct = nc.dram_tensor("ct", [D, VS], BF16, kind="ExternalInput").ap()
    # stats[i] i=0:a_s 1:a_t 2:w1 3:w2 ; [part p, btile, 1] -> row b=bt*128+p
    stats = nc.dram_tensor("stats", [4, 128, BT, 1], F32, kind="ExternalOutput").ap()

    hs_r = hs.rearrange("(kt p) b -> p kt b", p=128)
    ht_r = ht.rearrange("(kt p) b -> p kt b", p=128)
    cs_r = cs.rearrange("(kt p) v -> p kt v", p=128)
    ct_r = ct.rearrange("(kt p) v -> p kt v", p=128)

    with tile.TileContext(nc) as tc:
        with (
            tc.tile_pool(name="hpool", bufs=1) as hpool,
            tc.tile_pool(name="cpool", bufs=2) as cpool,
            tc.tile_pool(name="epool", bufs=3) as epool,
            tc.tile_pool(name="spool", bufs=3) as spool,
            tc.tile_pool(name="acc", bufs=1) as acc,
            tc.tile_pool(name="psum", bufs=2, space="PSUM") as psum,
        ):
            hs_sb = hpool.tile([128, KT, B], BF16, tag="hs")
            ht_sb = hpool.tile([128, KT, B], BF16, tag="ht")
            nc.sync.dma_start(out=hs_sb[:], in_=hs_r)
            nc.sync.dma_start(out=ht_sb[:], in_=ht_r)

            # per-(btile, megachunk) partial accumulators, col = bt*NMC+mc
            cols = [
                acc.tile([128, BT * NMC], F32, tag=f"cols{i}", name=f"cols{i}")
                for i in range(4)
            ]

            off = 0
            for mc, w in enumerate(MCS):
                cs_sb = cpool.tile([128, KT, MC_W], BF16, tag="cs")
                ct_sb = cpool.tile([128, KT, MC_W], BF16, tag="ct")
                nc.sync.dma_start(out=cs_sb[:, :, :w], in_=cs_r[:, :, off : off + w])
                nc.sync.dma_start(out=ct_sb[:, :, :w], in_=ct_r[:, :, off : off + w])

                for bt in range(BT):
                    ps = psum.tile([128, MC_W], F32, tag="ps")
                    pt = psum.tile([128, MC_W], F32, tag="pt")
                    b0 = bt * 128
                    for h0 in range(0, w, 512):
                        hw_ = min(512, w - h0)
                        for kt in range(KT):
                            nc.tensor.matmul(
                                ps[:, h0 : h0 + hw_],
                                hs_sb[:, kt, b0 : b0 + 128],
                                cs_sb[:, kt, h0 : h0 + hw_],
                                start=(kt == 0),
                                stop=(kt == KT - 1),
                            )
                        for kt in range(KT):
                            nc.tensor.matmul(
                                pt[:, h0 : h0 + hw_],
                                ht_sb[:, kt, b0 : b0 + 128],
                                ct_sb[:, kt, h0 : h0 + hw_],
                                start=(kt == 0),
                                stop=(kt == KT - 1),
                            )
                    idx = bt * NMC + mc
                    es = epool.tile([128, MC_W], BF16, tag="es")
                    et = epool.tile([128, MC_W], BF16, tag="et")
                    nc.scalar.activation(
                        es[:, :w], ps[:, :w], Exp, accum_out=cols[0][:, idx : idx + 1]
                    )
                    nc.scalar.activation(
                        et[:, :w], pt[:, :w], Exp, accum_out=cols[1][:, idx : idx + 1]
                    )
                    s1 = spool.tile([128, MC_W], BF16, tag="s1")
                    s2 = spool.tile([128, MC_W], BF16, tag="s2")
                    nc.vector.scalar_tensor_tensor(
                        s1[:, :w], et[:, :w], 1.0, pt[:, :w],
                        Alu.mult, Alu.mult, accum_out=cols[2][:, idx : idx + 1],
                    )
                    nc.vector.scalar_tensor_tensor(
                        s2[:, :w], et[:, :w], 1.0, ps[:, :w],
                        Alu.mult, Alu.mult, accum_out=cols[3][:, idx : idx + 1],
                    )
                off += w

            for i in range(4):
                red = acc.tile([128, BT, 1], F32, tag=f"red{i}", name=f"red{i}")
                nc.vector.tensor_reduce(
                    red[:],
                    cols[i].rearrange("p (bt mc) -> p bt mc", mc=NMC),
                    mybir.AxisListType.X,
                    Alu.add,
                )
                nc.sync.dma_start(out=stats[i], in_=red[:])

    nc.compile()
    return nc


def kernel(student_h, student_c, teacher_h, teacher_c, ce_coeff, kl_coeff, targets):
    bf = ml_dtypes.bfloat16
    hsT = np.ascontiguousarray(student_h.T).astype(bf)      # [D, B]
    htT = np.ascontiguousarray(teacher_h.T).astype(bf)
    csT = np.ascontiguousarray(student_c.T).astype(bf)      # [D, V]
    ctT = np.ascontiguousarray(teacher_c.T).astype(bf)

    if "nc" not in _COMPILED:
        _COMPILED["nc"] = _build()
    nc = _COMPILED["nc"]

    in_maps = []
    for k in range(NCORES):
        sl = slice(k * VS, (k + 1) * VS)
        in_maps.append(
            {
                "hs": hsT,
                "ht": htT,
                "cs": np.ascontiguousarray(csT[:, sl]),
                "ct": np.ascontiguousarray(ctT[:, sl]),
            }
        )
    res = run_bass_kernel_spmd(nc, in_maps, core_ids=list(range(NCORES)))

    a_s = np.zeros(B, np.float64)
    a_t = np.zeros(B, np.float64)
    w1 = np.zeros(B, np.float64)
    w2 = np.zeros(B, np.float64)
    for k in range(NCORES):
        st = res.results[k]["stats"].astype(np.float64)  # [4,128,BT,1]
        a_s += st[0, :, :, 0].T.reshape(B)               # b = bt*128 + p
        a_t += st[1, :, :, 0].T.reshape(B)
        w1 += st[2, :, :, 0].T.reshape(B)
        w2 += st[3, :, :, 0].T.reshape(B)

    s_lse = np.log(a_s)
    t_lse = np.log(a_t)
    h64 = student_h.astype(np.float64)
    c_tgt = student_c[np.asarray(targets)].astype(np.float64)  # [B, D]
    tgt_logit = np.einsum("bd,bd->b", h64, c_tgt)
    ce = s_lse - tgt_logit
    kl = (w1 - w2) / a_t - t_lse + s_lse
    loss = np.sum(
        ce_coeff.astype(np.float64) * ce + kl_coeff.astype(np.float64) * kl
    )
    return np.float32(loss)


# revision 5
# speedup vs baseline: 99.1452x; 99.1452x over previous
"""Fused LinearCrossEntropyKL loss on 8 Trainium2 cores (vocab-parallel).

loss = sum_b ce_c[b]*(s_lse[b] - s_logit[b,tgt[b]]) + kl_c[b]*KL(t||s)[b]

Device (per core, vocab shard Vs=6288):
  a_s[b] = sum_v exp(s_logit),  a_t[b] = sum_v exp(t_logit)
  w1[b]  = sum_v exp(t)*t,      w2[b]  = sum_v exp(t)*s
Host combine: s_lse = log(sum_k a_s), u = w1-w2,
  KL = u/a_t - t_lse + s_lse, ce = s_lse - <h_b, c_tgt_b>  (host fp64 dot)
No max-stabilizer: inputs are scaled 1/sqrt(D) so |logit| < ~0.5 and
exp() is safely in range.
"""

import sys

sys.path.insert(0, "/opt/trn_rl_repo")

import numpy as np
import ml_dtypes

import concourse.bacc as bacc
import concourse.bass as bass
import concourse.mybir as mybir
from concourse import tile
from concourse.bass_utils import run_bass_kernel_spmd

B, D, V = 2048, 512, 50304
NCORES = 8
VS = V // NCORES            # 6288 per core
KT = D // 128               # 4 k-tiles
BT = B // 128               # 16 b-tiles
MC_W = 1024                 # megachunk width (2 PSUM banks)
MCS = [MC_W] * (VS // MC_W) + ([VS % MC_W] if VS % MC_W else [])  # 6x1024+144
NMC = len(MCS)

F32 = mybir.dt.float32
BF16 = mybir.dt.bfloat16
Exp = mybir.ActivationFunctionType.Exp
Alu = mybir.AluOpType

_COMPILED = {}


def _build(reps=None):
    nc = bacc.Bacc(
        "TRN2", target_bir_lowering=False, debug=False, num_devices=NCORES
    )
    hs = nc.dram_tensor("hs", [D, B], BF16, kind="ExternalInput").ap()
    ht = nc.dram_tensor("ht", [D, B], BF16, kind="ExternalInput").ap()
    cs = nc.dram_tensor("cs", [D, VS], BF16, kind="ExternalInput").ap()
    ct = nc.dram_tensor("ct", [D, VS], BF16, kind="ExternalInput").ap()
    # stats[i] i=0:a_s 1:a_t 2:w1 3:w2 ; [part p, btile, 1] -> row b=bt*128+p
    stats = nc.dram_tensor("stats", [4, 128, BT, 1], F32, kind="ExternalOutput").ap()

    hs_r = hs.rearrange("(kt p) b -> p kt b", p=128)
    ht_r = ht.rearrange("(kt p) b -> p kt b", p=128)
    cs_r = cs.rearrange("(kt p) v -> p kt v", p=128)
    ct_r = ct.rearrange("(kt p) v -> p kt v", p=128)

    import contextlib

    with tile.TileContext(nc) as tc:
        with (
            tc.For_i(0, reps, 1) if reps else contextlib.nullcontext(),
            tc.tile_pool(name="hpool", bufs=1) as hpool,
            tc.tile_pool(name="cpool", bufs=2) as cpool,
            tc.tile_pool(name="epool", bufs=3) as epool,
            tc.tile_pool(name="spool", bufs=3) as spool,
            tc.tile_pool(name="acc", bufs=1) as acc,
            tc.tile_pool(name="psum", bufs=2, space="PSUM") as psum,
        ):
            hs_sb = hpool.tile([128, KT, B], BF16, tag="hs")
            ht_sb = hpool.tile([128, KT, B], BF16, tag="ht")
            nc.sync.dma_start(out=hs_sb[:], in_=hs_r)
            nc.sync.dma_start(out=ht_sb[:], in_=ht_r)

            # per-(btile, megachunk) partial accumulators, col = bt*NMC+mc
            cols = [
                acc.tile([128, BT * NMC], F32, tag=f"cols{i}", name=f"cols{i}")
                for i in range(4)
            ]

            off = 0
            for mc, w in enumerate(MCS):
                cs_sb = cpool.tile([128, KT, MC_W], BF16, tag="cs")
                ct_sb = cpool.tile([128, KT, MC_W], BF16, tag="ct")
                nc.sync.dma_start(out=cs_sb[:, :, :w], in_=cs_r[:, :, off : off + w])
                nc.sync.dma_start(out=ct_sb[:, :, :w], in_=ct_r[:, :, off : off + w])

                for bt in range(BT):
                    ps = psum.tile([128, MC_W], F32, tag="ps")
                    pt = psum.tile([128, MC_W], F32, tag="pt")
                    b0 = bt * 128
                    for h0 in range(0, w, 512):
                        hw_ = min(512, w - h0)
                        for kt in range(KT):
                            nc.tensor.matmul(
                                ps[:, h0 : h0 + hw_],
                                hs_sb[:, kt, b0 : b0 + 128],
                                cs_sb[:, kt, h0 : h0 + hw_],
                                start=(kt == 0),
                                stop=(kt == KT - 1),
                            )
                        for kt in range(KT):
                            nc.tensor.matmul(
                                pt[:, h0 : h0 + hw_],
                                ht_sb[:, kt, b0 : b0 + 128],
                                ct_sb[:, kt, h0 : h0 + hw_],
                                start=(kt == 0),
                                stop=(kt == KT - 1),
                            )
                    idx = bt * NMC + mc
                    es = epool.tile([128, MC_W], BF16, tag="es")
                    et = epool.tile([128, MC_W], BF16, tag="et")
                    nc.scalar.activation(
                        es[:, :w], ps[:, :w], Exp, accum_out=cols[0][:, idx : idx + 1]
                    )
                    nc.scalar.activation(
                        et[:, :w], pt[:, :w], Exp, accum_out=cols[1][:, idx : idx + 1]
                    )
                    s1 = spool.tile([128, MC_W], BF16, tag="s1")
                    s2 = spool.tile([128, MC_W], BF16, tag="s2")
                    nc.vector.scalar_tensor_tensor(
                        s1[:, :w], et[:, :w], 1.0, pt[:, :w],
                        Alu.mult, Alu.mult, accum_out=cols[2][:, idx : idx + 1],
                    )
                    nc.vector.scalar_tensor_tensor(
                        s2[:, :w], et[:, :w], 1.0, ps[:, :w],
                        Alu.mult, Alu.mult, accum_out=cols[3][:, idx : idx + 1],
                    )
                off += w

            for i in range(4):
                red = acc.tile([128, BT, 1], F32, tag=f"red{i}", name=f"red{i}")
                nc.vector.tensor_reduce(
                    red[:],
                    cols[i].rearrange("p (bt mc) -> p bt mc", mc=NMC),
                    mybir.AxisListType.X,
                    Alu.add,
                )
                nc.sync.dma_start(out=stats[i], in_=red[:])

    nc.compile()
    return nc


def kernel(student_h, student_c, teacher_h, teacher_c, ce_coeff, kl_coeff, targets):
    bf = ml_dtypes.bfloat16
    hsT = np.ascontiguousarray(student_h.T).astype(bf)      # [D, B]
    htT = np.ascontiguousarray(teacher_h.T).astype(bf)
    csT = np.ascontiguousarray(student_c.T).astype(bf)      # [D, V]
    ctT = np.ascontiguousarray(teacher_c.T).astype(bf)

    if "nc" not in _COMPILED:
        _COMPILED["nc"] = _build()
    nc = _COMPILED["nc"]

    in_maps = []
    for k in range(NCORES):
        sl = slice(k * VS, (k + 1) * VS)
        in_maps.append(
            {
                "hs": hsT,
                "ht": htT,
                "cs": np.ascontiguousarray(csT[:, sl]),
                "ct": np.ascontiguousarray(ctT[:, sl]),
            }
        )
    res = run_bass_kernel_spmd(nc, in_maps, core_ids=list(range(NCORES)))

    a_s = np.zeros(B, np.float64)
    a_t = np.zeros(B, np.float64)
    w1 = np.zeros(B, np.float64)
    w2 = np.zeros(B, np.float64)
    for k in range(NCORES):
        st = res.results[k]["stats"].astype(np.float64)  # [4,128,BT,1]
        a_s += st[0, :, :, 0].T.reshape(B)               # b = bt*128 + p
        a_t += st[1, :, :, 0].T.reshape(B)
        w1 += st[2, :, :, 0].T.reshape(B)
        w2 += st[3, :, :, 0].T.reshape(B)

    s_lse = np.log(a_s)
    t_lse = np.log(a_t)
    h64 = student_h.astype(np.float64)
    c_tgt = student_c[np.asarray(targets)].astype(np.float64)  # [B, D]
    tgt_logit = np.einsum("bd,bd->b", h64, c_tgt)
    ce = s_lse - tgt_logit
    kl = (w1 - w2) / a_t - t_lse + s_lse
    loss = np.sum(
        ce_coeff.astype(np.float64) * ce + kl_coeff.astype(np.float64) * kl
    )
    return np.float32(loss)
